# revision 1
# baseline (speedup 1.0000x reference)
"""Trainium2 Bass kernel for nn_Attention_77824807403911 (sparse_attention).

Math (per batch element, no softmax => associativity):
    q = x @ Wq^T + bq ; v = x @ Wv^T + bv          [1024, 256]
    rq = rope(q) ; rv = rope(v)
    per head h (16 heads, hd=16):  att_h = rq_h @ (rq_h^T @ rv_h) / 4
    out = att @ Wo^T + bo

Instead of the 1024x1024 score matrix we compute the 16x16 Gram per head
(64x fewer flops), realized as a full 256x256 Gram masked to the
block-diagonal, folded with Wo into a single per-batch [256,256] weight:
    F[e,f]  = sum_s rv[s,e] rq[s,f]       (Gram, transposed blocks)
    BDT     = F .* blockmask
    W2[f,o] = sum_e BDT[e,f] * Wo[o,e] / 4
    outT    = W2^T @ rqT + bo             ([256, 1024])

Sharding: data-parallel over batch, 1 element per core, no collectives.

Layout: channels permuted so rope pairs split into even/odd partition
halves; activations kept transposed ([chan, seq]) so rope trig varies
along the free dim and biases are per-partition (ACT-friendly). Natural
layout (for the Gram contraction over s) is produced by whole-chunk
DMA xbar transposes with 3D tiled output APs. Compute dtype bf16
(fp32 PSUM accumulation), f32 output.
"""

import numpy as np
import ml_dtypes

import concourse.bass as bass
import concourse.bacc as bacc
import concourse.tile as tile
from concourse import mybir
from concourse.bass_utils import run_bass_kernel_spmd

B, S, D, H, HD = 8, 1024, 256, 16, 16
N_CORES = 8
BF16 = mybir.dt.bfloat16
F32 = mybir.dt.float32

# channel permutation: [evens of pairs 0-63 (theta=1), evens of pairs 64-127
# (theta=1e-4), odds of pairs 0-63, odds of pairs 64-127]
PERM = np.concatenate(
    [np.arange(0, 128, 2), np.arange(128, 256, 2),
     np.arange(1, 128, 2), np.arange(129, 256, 2)]
)


def _host_tables():
    p = np.arange(128)
    theta = np.where(p < 64, 1.0, 1e-4)
    s = np.arange(S, dtype=np.float64) + 1.0
    ang = theta[:, None] * s[None, :]
    sin = np.sin(ang).astype(ml_dtypes.bfloat16)
    cos = np.cos(ang).astype(ml_dtypes.bfloat16)
    a = np.arange(256)
    headp = (a % 128) // 8
    mask = (headp[:, None] == headp[None, :]).astype(ml_dtypes.bfloat16)
    return sin, cos, mask


def build_kernel():
    nc = bacc.Bacc()
    xT = nc.declare_dram_parameter("xT", [D, S], BF16, isOutput=False)
    # wbig columns: [wqt | wvt | wot | mask], each [256, 256]
    wbig = nc.declare_dram_parameter("wbig", [D, 4 * D + 3], BF16, isOutput=False)
    # trig columns: [sin | cos], each [128, 1024]
    trig = nc.declare_dram_parameter("trig", [128, 2 * S], BF16, isOutput=False)
    outT = nc.declare_dram_parameter("outT", [D, S], F32, isOutput=True)

    with tile.TileContext(nc) as tc:
        _body(tc, xT, wbig, trig, outT)
    nc.compile()
    return nc


def _body(tc, xT, wbig, trig, outT):
    nc = tc.nc
    NS = 2          # s chunks of 512 for matmul streaming
    SC = S // NS    # 512

    with (
        tc.tile_pool(name="const", bufs=1) as cpool,
        tc.tile_pool(name="acts", bufs=1) as apool,
        tc.tile_pool(name="psum", bufs=4, space="PSUM") as pp,
        tc.tile_pool(name="outp", bufs=4) as opool,
    ):
        # ---- constant loads; 3 input DMAs total ----
        trig_sb = cpool.tile([128, 2 * S], BF16, tag="trig", name="trig_sb")
        nc.scalar.dma_start(trig_sb[:], trig[:])
        sin_sb = trig_sb[:, 0:S]
        cos_sb = trig_sb[:, S:2 * S]

        # PE warm-up: garbage matmuls release the HAM clock gate (3.4us of
        # activity -> 2.4GHz) while the real inputs stream in. scratch is
        # memset early in the preamble, so these run before any input lands.
        scratch = cpool.tile([128, 512], BF16, tag="scratch", name="scratch")
        nc.gpsimd.memset(scratch[:], 0.25)
        warm_ps = pp.tile([128, 512], F32, tag="warm", bufs=1, name="warm_ps")
        for wi in range(8):
            nc.tensor.matmul(warm_ps[:], scratch[:, 0:128], scratch[:],
                             start=True, stop=True, skip_group_check=True)

        xT_sb, w_sb = [], []
        for cc in range(2):
            t = cpool.tile([128, S], BF16, tag=f"xT{cc}", name=f"xT{cc}")
            xT_sb.append(t)
        for cc in range(2):
            t = cpool.tile([128, 4 * D + 3], BF16, tag=f"wbig{cc}", name=f"wbig{cc}")
            w_sb.append(t)
        nc.sync.dma_start(xT_sb[0][:], xT[0:128, :])
        nc.scalar.dma_start(xT_sb[1][:], xT[128:256, :])
        nc.sync.dma_start(w_sb[0][:], wbig[0:128, :])
        nc.scalar.dma_start(w_sb[1][:], wbig[128:256, :])

        def wslice(idx, cc, col0, ncol):
            # weight idx (0=wqt,1=wvt,2=wot,3=mask), chunk cc, cols [col0, col0+ncol)
            return w_sb[cc][:, idx * D + col0: idx * D + col0 + ncol]

        def bias_ap(idx, cc):
            return w_sb[cc][:, 4 * D + idx: 4 * D + idx + 1]

        # transposed-permuted activations, 2 chunks of [128, 1024]
        def act2(tag, width=S, dtype=BF16):
            return [apool.tile([128, width], dtype, tag=f"{tag}{cc}", name=f"{tag}{cc}")
                    for cc in range(2)]

        qT = act2("qT")
        vT = act2("vT")
        rqT = act2("rqT")
        rvT = act2("rvT")
        # natural layout [s=128, chan 256] x 8 s-tiles packed along free dim
        rq_nat = apool.tile([128, 8 * D], BF16, tag="rq_nat")
        rv_nat = apool.tile([128, 8 * D], BF16, tag="rv_nat")

        # ---- projections: tT[a, s] = sum_d w[d, a] x[d, s] (+bias at evict) ----
        def project(widx, bidx, dstT):
            for ac in range(2):          # output chan chunk (partition)
                for sc in range(NS):     # s chunk
                    ps = pp.tile([128, SC], F32, tag="mm", bufs=5, name="proj_ps")
                    for dc in range(2):  # contraction chunk
                        nc.tensor.matmul(
                            ps[:],
                            wslice(widx, dc, ac * 128, 128),
                            xT_sb[dc][:, sc * SC:(sc + 1) * SC],
                            start=(dc == 0), stop=(dc == 1),
                        )
                    nc.scalar.activation(
                        dstT[ac][:, sc * SC:(sc + 1) * SC],
                        ps[:],
                        mybir.ActivationFunctionType.Identity,
                        bias=bias_ap(bidx, ac),
                    )

        project(0, 0, qT)   # q
        project(1, 1, vT)   # v

        # ---- rope (transposed layout): rE = E*sin - O*cos ; rO = E*cos + O*sin ----
        def rope(srcT, dstT, tmp_tag):
            E, O = srcT[0][:], srcT[1][:]
            t1 = opool.tile([128, S], BF16, tag=tmp_tag + "1", name=tmp_tag + "1")
            t2 = opool.tile([128, S], BF16, tag=tmp_tag + "2", name=tmp_tag + "2")
            nc.vector.tensor_tensor(t1[:], E, sin_sb, mybir.AluOpType.mult)
            nc.vector.tensor_tensor(t2[:], O, cos_sb, mybir.AluOpType.mult)
            nc.vector.tensor_tensor(dstT[0][:], t1[:], t2[:], mybir.AluOpType.subtract)
            t3 = opool.tile([128, S], BF16, tag=tmp_tag + "3", name=tmp_tag + "3")
            t4 = opool.tile([128, S], BF16, tag=tmp_tag + "4", name=tmp_tag + "4")
            nc.vector.tensor_tensor(t3[:], E, cos_sb, mybir.AluOpType.mult)
            nc.vector.tensor_tensor(t4[:], O, sin_sb, mybir.AluOpType.mult)
            nc.vector.tensor_tensor(dstT[1][:], t3[:], t4[:], mybir.AluOpType.add)

        rope(qT, rqT, "rq_tmp")
        rope(vT, rvT, "rv_tmp")

        # keep the PE clock-gate open across the rope gap (deps on qT/vT
        # force these after the projections, filling the otherwise-idle span)
        for wi in range(12):
            srcv = vT[wi % 2]
            nc.tensor.matmul(warm_ps[:], srcv[:, 0:128], srcv[:, 0:512],
                             start=True, stop=True, skip_group_check=True)

        # ---- whole-chunk transposes to natural layout (3D tiled out AP) ----
        # out[p, st, j] = in[j, st*128 + p]  => nat[s=st*128+p, chan cc*128+j]
        rq_nat3 = rq_nat[:].rearrange("p (st c) -> p st c", c=D)
        rv_nat3 = rv_nat[:].rearrange("p (st c) -> p st c", c=D)
        for cc in range(2):
            nc.sync.dma_start(
                rq_nat3[:, :, cc * 128:(cc + 1) * 128], rqT[cc][:], transpose=True)
            nc.scalar.dma_start(
                rv_nat3[:, :, cc * 128:(cc + 1) * 128], rvT[cc][:], transpose=True)

        # ---- Gram: Hm[e, f] = sum_s rv[s, e] rq[s, f]; mask -> BDT (bf16) ----
        bdt = act2("bdt", width=D)
        for ec in range(2):
            ps = pp.tile([128, D], F32, tag="sm", bufs=2, name="gram_ps")
            for st in range(8):
                nc.tensor.matmul(
                    ps[:],
                    rv_nat[:, st * D + ec * 128: st * D + (ec + 1) * 128],
                    rq_nat[:, st * D: (st + 1) * D],
                    start=(st == 0), stop=(st == 7),
                )
            nc.vector.tensor_tensor(
                bdt[ec][:], ps[:], wslice(3, ec, 0, D), mybir.AluOpType.mult)

        # ---- W2[f, o] = sum_e BDT[e, f] wot[e, o]  (scaled by 1/4 at evict) ----
        w2 = act2("w2", width=D)
        for fc in range(2):
            ps = pp.tile([128, D], F32, tag="sm", bufs=2, name="w2_ps")
            for ec in range(2):
                nc.tensor.matmul(
                    ps[:],
                    bdt[ec][:, fc * 128:(fc + 1) * 128],
                    wslice(2, ec, 0, D),
                    start=(ec == 0), stop=(ec == 1),
                )
            nc.scalar.activation(
                w2[fc][:], ps[:],
                mybir.ActivationFunctionType.Copy, scale=0.25)

        # ---- final: outT[o, s] = sum_f W2[f, o] rqT[f, s] + bo ----
        for oc in range(2):
            for sc in range(NS):
                ps = pp.tile([128, SC], F32, tag="mm", bufs=5, name="fin_ps")
                for fc in range(2):
                    nc.tensor.matmul(
                        ps[:],
                        w2[fc][:, oc * 128:(oc + 1) * 128],
                        rqT[fc][:, sc * SC:(sc + 1) * SC],
                        start=(fc == 0), stop=(fc == 1),
                    )
                ot = opool.tile([128, SC], F32, tag="out_sb", name="out_sb")
                nc.scalar.activation(
                    ot[:], ps[:],
                    mybir.ActivationFunctionType.Identity,
                    bias=bias_ap(2, oc),
                )
                eng = nc.scalar if (oc + sc) % 2 == 0 else nc.sync
                eng.dma_start(
                    outT[oc * 128:(oc + 1) * 128, sc * SC:(sc + 1) * SC], ot[:])


_NC_CACHE = None


def _get_nc():
    global _NC_CACHE
    if _NC_CACHE is None:
        _NC_CACHE = build_kernel()
    return _NC_CACHE


def make_in_maps(x, wq_w, wq_b, wv_w, wv_b, wo_w, wo_b):
    sin, cos, mask = _host_tables()
    wq_p = np.ascontiguousarray(wq_w[PERM].T).astype(ml_dtypes.bfloat16)   # [d, a]
    wv_p = np.ascontiguousarray(wv_w[PERM].T).astype(ml_dtypes.bfloat16)
    wo_p = np.ascontiguousarray(wo_w[:, PERM].T).astype(ml_dtypes.bfloat16)  # [a(e), o]
    bias3 = np.stack([wq_b[PERM], wv_b[PERM], wo_b], axis=1).astype(ml_dtypes.bfloat16)
    wbig = np.ascontiguousarray(
        np.concatenate([wq_p, wv_p, wo_p, mask, bias3], axis=1))
    trig = np.ascontiguousarray(np.concatenate([sin, cos], axis=1))
    in_maps = []
    for b in range(B):
        in_maps.append({
            "xT": np.ascontiguousarray(x[b].T).astype(ml_dtypes.bfloat16),
            "wbig": wbig, "trig": trig,
        })
    return in_maps


TRACE = False
RUN_KWARGS = {}
LAST_RESULT = None


def kernel(x, wq_w, wq_b, wk_w, wk_b, wv_w, wv_b, wo_w, wo_b):
    global LAST_RESULT
    x = np.asarray(x, dtype=np.float32)
    in_maps = make_in_maps(x, np.asarray(wq_w, np.float32), np.asarray(wq_b, np.float32),
                           np.asarray(wv_w, np.float32), np.asarray(wv_b, np.float32),
                           np.asarray(wo_w, np.float32), np.asarray(wo_b, np.float32))
    nc = _get_nc()
    res = run_bass_kernel_spmd(nc, in_maps, core_ids=list(range(N_CORES)),
                               trace=TRACE, **RUN_KWARGS)
    LAST_RESULT = res
    outs = [np.ascontiguousarray(res.results[b]["outT"].T) for b in range(B)]
    return np.stack(outs).astype(np.float32)

